# revision 4
# baseline (speedup 1.0000x reference)
"""Trainium2 Bass kernel for nn_NewAttention_55344948576827.

Math: reference computes
    v   = x @ W1.T                      (x: [B,S,E], W1: [E,E])
    att = w_pos @ v  (per head)         (w_pos[q,k] = c*exp(-0.5*(k-q)^2), [S,S])
    out = att @ W2.T
The positional weights are data independent, identical for every head and
channel, and decay below fp32 noise for |k-q| > 8 — so the [S,S] matmul is a
17-tap 1D convolution along S.  The conv acts on the S axis only and both
projections act on the E axis only, so they commute:
    out = conv_S(x) @ (W2 @ W1).T
One fused weight matrix, one conv.

Device mapping (per core):
  - conv_S as PE matmuls against a constant banded [272, 256] matrix; with x
    tiles as the stationary operand this *also* transposes x (e lands on
    partitions), exactly what the main matmul needs.
  - WcT = W1.T @ W2.T computed on-device once per core (4 accumulating
    matmuls; W2 is shipped pre-transposed from the host — a zero-FLOP
    relayout, like the halo prep).
  - main: out[s,f] = sum_e xcT[e,s] * WcT[e,f], 4 accumulating matmuls per
    128-row output tile.

Sharding: B*S = 16384 rows split 8 ways -> 2048 rows/core (half a batch, so
the conv never crosses a core's slice except through an 8-row halo baked into
the shipped input).  No collectives.
"""

import os
import threading
from contextlib import ExitStack

import numpy as np

import concourse.bass as bass
import concourse.tile as tile
from concourse import bacc, mybir
from concourse.bass_utils import run_bass_kernel_spmd
# ---------------------------------------------------------------- constants
B, S, E = 4, 4096, 512
N_CORES = 8
ROWS = (B * S) // N_CORES          # 2048 rows per core
R = 8                              # gaussian band radius (17 taps)
XROWS = ROWS + 2 * R               # 2064 valid rows shipped per core
XPAD = 2176                        # padded to 17 tiles of 128
N_XT = XPAD // 128                 # 17 x tiles
N_BLK = ROWS // 256                # 8 conv blocks of 256 output rows
N_ROWT = ROWS // 128               # 16 output row tiles

# matmul dtype: "f32r" (full fp32 data, relaxed-precision PE mode, ~1.5e-4) or
# "bf16" (inputs quantized host-side, ~4e-3)
DTYPE = os.environ.get("KERNEL_DTYPE", "f32r")

_lock = threading.Lock()
_cache = {}


def _band_matrix() -> np.ndarray:
    """Constant [272, 256] band matrix: band[k, j] = g(k - j - R)."""
    coef = np.float32(1.0 / np.sqrt(2.0 * np.pi))
    band = np.zeros((256 + 2 * R, 256), dtype=np.float64)
    k = np.arange(256 + 2 * R)[:, None]
    j = np.arange(256)[None, :]
    d = k - j - R
    mask = np.abs(d) <= R
    band[mask] = (coef * np.exp(-0.5 * d.astype(np.float64) ** 2))[mask]
    return band.astype(np.float32)


def _build(dtype_flag: str):
    mdt = {"f32r": mybir.dt.float32r, "bf16": mybir.dt.bfloat16}[dtype_flag]
    f32 = mybir.dt.float32

    nc = bacc.Bacc("TRN2", target_bir_lowering=False, debug=False,
                   num_devices=N_CORES)

    xd = nc.dram_tensor("x", [XPAD, E], mdt, kind="ExternalInput").ap()
    w1d = nc.dram_tensor("w1", [E, E], mdt, kind="ExternalInput").ap()
    w2d = nc.dram_tensor("w2t", [E, E], mdt, kind="ExternalInput").ap()
    bdd = nc.dram_tensor("band", [256 + 2 * R, 256], mdt,
                         kind="ExternalInput").ap()
    od = nc.dram_tensor("out", [ROWS, E], f32, kind="ExternalOutput").ap()

    with tile.TileContext(nc) as tc, ExitStack() as ctx:
        xp = ctx.enter_context(tc.tile_pool(name="xp", bufs=N_XT))
        w1p = ctx.enter_context(tc.tile_pool(name="w1p", bufs=4))
        w2tp = ctx.enter_context(tc.tile_pool(name="w2tp", bufs=4))
        wctp = ctx.enter_context(tc.tile_pool(name="wctp", bufs=4))
        bp = ctx.enter_context(tc.tile_pool(name="bp", bufs=1))
        cvp = ctx.enter_context(tc.tile_pool(name="cvp", bufs=8))
        outp = ctx.enter_context(tc.tile_pool(name="outp", bufs=3))
        psA = ctx.enter_context(tc.tile_pool(name="psA", bufs=3, space="PSUM"))
        psB = ctx.enter_context(tc.tile_pool(name="psB", bufs=3, space="PSUM"))

        # ------------------------------------------------ load inputs
        xt = []
        for u in range(N_XT):
            t = xp.tile([128, E], mdt, tag="xt", name=f"xt{u}")
            nc.sync.dma_start(out=t[:], in_=xd[128 * u:128 * u + 128, :])
            xt.append(t)

        w1t = []
        w2T = []
        for i in range(4):
            t1 = w1p.tile([128, E], mdt, tag="w1", name=f"w1_{i}")
            nc.sync.dma_start(out=t1[:], in_=w1d[128 * i:128 * i + 128, :])
            w1t.append(t1)
            t2 = w2tp.tile([128, E], mdt, tag="w2T", name=f"w2T_{i}")
            nc.sync.dma_start(out=t2[:], in_=w2d[128 * i:128 * i + 128, :])
            w2T.append(t2)

        b0 = bp.tile([128, 256], mdt, tag="b0")
        nc.sync.dma_start(out=b0[:], in_=bdd[0:128, :])
        b1 = bp.tile([128, 256], mdt, tag="b1")
        nc.sync.dma_start(out=b1[:], in_=bdd[128:256, :])
        b2 = bp.tile([2 * R, 256], mdt, tag="b2")
        nc.sync.dma_start(out=b2[:], in_=bdd[256:256 + 2 * R, :])

        # ------------------------------------------------ WcT[e, f] = sum_m W1[m,e] W2T[m,f]
        wcT = [wctp.tile([128, E], mdt, tag="wcT", name=f"wcT_{i}") for i in range(4)]
        for ei in range(4):
            pw = psB.tile([128, E], f32, tag="psB", name="psB_t")
            for mi in range(4):
                nc.tensor.matmul(
                    pw[:],
                    w1t[mi][:, 128 * ei:128 * ei + 128],
                    w2T[mi][:],
                    start=(mi == 0),
                    stop=(mi == 3),
                )
            nc.vector.tensor_copy(wcT[ei][:], pw[:])

        # ------------------------------------------------ main loop
        for t in range(N_BLK):
            # conv block t: xcT[e, 256t + j] = sum_k x[k, e] band[k - 256t, j]
            xcT = []
            for ei in range(4):
                pc = psA.tile([128, 256], f32, tag="psA", name="psA_t")
                es = slice(128 * ei, 128 * ei + 128)
                nc.tensor.matmul(pc[:], xt[2 * t][:, es], b0[:],
                                 start=True, stop=False)
                nc.tensor.matmul(pc[:], xt[2 * t + 1][:, es], b1[:],
                                 start=False, stop=False)
                nc.tensor.matmul(pc[:], xt[2 * t + 2][0:2 * R, es], b2[:],
                                 start=False, stop=True)
                ct = cvp.tile([128, 256], mdt, tag="xcT", name=f"xcT_{t}_{ei}")
                nc.vector.tensor_copy(ct[:], pc[:])
                xcT.append(ct)

            for q in range(2):           # two 128-row output tiles per block
                r = 2 * t + q
                po = psB.tile([128, E], f32, tag="psB", name="psB_t")
                ss = slice(128 * q, 128 * q + 128)
                for ei in range(4):
                    nc.tensor.matmul(po[:], xcT[ei][:, ss], wcT[ei][:],
                                     start=(ei == 0), stop=(ei == 3))
                ot = outp.tile([128, E], f32, tag="ot", name=f"ot{r}")
                nc.vector.tensor_copy(ot[:], po[:])
                nc.sync.dma_start(out=od[128 * r:128 * r + 128, :], in_=ot[:])

    nc.compile()
    return nc


def _get_nc(dtype_flag: str):
    with _lock:
        if dtype_flag not in _cache:
            _cache[dtype_flag] = _build(dtype_flag)
        return _cache[dtype_flag]


def _np_dtype(dtype_flag: str):
    if dtype_flag == "bf16":
        import ml_dtypes
        return ml_dtypes.bfloat16
    return np.float32


def kernel(inputs: np.ndarray, input_weights: np.ndarray,
           output_weight: np.ndarray) -> np.ndarray:
    x = np.ascontiguousarray(np.asarray(inputs, dtype=np.float32))
    w1 = np.asarray(input_weights, dtype=np.float32)
    w2 = np.asarray(output_weight, dtype=np.float32)
    assert x.shape == (B, S, E) and w1.shape == (E, E) and w2.shape == (E, E)

    nc = _get_nc(DTYPE)
    ndt = _np_dtype(DTYPE)

    w1s = np.ascontiguousarray(w1.astype(ndt))
    w2s = np.ascontiguousarray(w2.T.astype(ndt))   # ship W2 pre-transposed
    band = np.ascontiguousarray(_band_matrix().astype(ndt))

    halves = S // 2                  # rows per core within a batch
    in_maps = []
    for c in range(N_CORES):
        b, half = divmod(c, 2)
        s0 = half * halves
        s1 = s0 + halves
        xc = np.zeros((XPAD, E), dtype=ndt)
        xc[R:R + ROWS] = x[b, s0:s1].astype(ndt)
        if s0 > 0:
            xc[0:R] = x[b, s0 - R:s0].astype(ndt)
        if s1 < S:
            xc[R + ROWS:R + ROWS + R] = x[b, s1:s1 + R].astype(ndt)
        in_maps.append({"x": xc, "w1": w1s, "w2t": w2s, "band": band})

    res = run_bass_kernel_spmd(nc, in_maps, core_ids=list(range(N_CORES)))

    out = np.empty((B, S, E), dtype=np.float32)
    for c in range(N_CORES):
        b, half = divmod(c, 2)
        s0 = half * halves
        out[b, s0:s0 + halves] = res.results[c]["out"]
    return out


# revision 5
# speedup vs baseline: 1.1057x; 1.1057x over previous
"""Trainium2 Bass kernel for nn_NewAttention_55344948576827.

Math: reference computes
    v   = x @ W1.T                      (x: [B,S,E], W1: [E,E])
    att = w_pos @ v  (per head)         (w_pos[q,k] = c*exp(-0.5*(k-q)^2), [S,S])
    out = att @ W2.T
The positional weights are data independent, identical for every head and
channel, and decay below fp32 noise for |k-q| > 8 — so the [S,S] matmul is a
17-tap 1D convolution along S.  The conv acts on the S axis only and both
projections act on the E axis only, so they commute:
    out = conv_S(x) @ (W2 @ W1).T
One fused weight matrix, one conv.

Device mapping (per core):
  - conv_S as PE matmuls against a constant banded [272, 256] matrix; with x
    tiles as the stationary operand this *also* transposes x (e lands on
    partitions), exactly what the main matmul needs.
  - WcT = W1.T @ W2.T computed on-device once per core (4 accumulating
    matmuls; W2 is shipped pre-transposed from the host — a zero-FLOP
    relayout, like the halo prep).
  - main: out[s,f] = sum_e xcT[e,s] * WcT[e,f], 4 accumulating matmuls per
    128-row output tile.

Sharding: B*S = 16384 rows split 8 ways -> 2048 rows/core (half a batch, so
the conv never crosses a core's slice except through an 8-row halo baked into
the shipped input).  No collectives.
"""

import os
import threading
from contextlib import ExitStack

import numpy as np

import concourse.bass as bass
import concourse.tile as tile
from concourse import bacc, mybir
from concourse.bass_utils import run_bass_kernel_spmd
# ---------------------------------------------------------------- constants
B, S, E = 4, 4096, 512
N_CORES = 8
ROWS = (B * S) // N_CORES          # 2048 rows per core
R = 8                              # gaussian band radius (17 taps)
XROWS = ROWS + 2 * R               # 2064 valid rows shipped per core
XPAD = 2176                        # padded to 17 tiles of 128
N_XT = XPAD // 128                 # 17 x tiles
N_BLK = ROWS // 256                # 8 conv blocks of 256 output rows
N_ROWT = ROWS // 128               # 16 output row tiles

# matmul dtype: "f32r" (full fp32 data, relaxed-precision PE mode, ~1.5e-4) or
# "bf16" (inputs quantized host-side, ~4e-3)
DTYPE = os.environ.get("KERNEL_DTYPE", "f32r")

_lock = threading.Lock()
_cache = {}


def _band_matrix() -> np.ndarray:
    """Constant [272, 256] band matrix: band[k, j] = g(k - j - R)."""
    coef = np.float32(1.0 / np.sqrt(2.0 * np.pi))
    band = np.zeros((256 + 2 * R, 256), dtype=np.float64)
    k = np.arange(256 + 2 * R)[:, None]
    j = np.arange(256)[None, :]
    d = k - j - R
    mask = np.abs(d) <= R
    band[mask] = (coef * np.exp(-0.5 * d.astype(np.float64) ** 2))[mask]
    return band.astype(np.float32)


def _build(dtype_flag: str):
    mdt = {"f32r": mybir.dt.float32r, "bf16": mybir.dt.bfloat16}[dtype_flag]
    f32 = mybir.dt.float32

    nc = bacc.Bacc("TRN2", target_bir_lowering=False, debug=False,
                   num_devices=N_CORES)

    xd = nc.dram_tensor("x", [XPAD, E], mdt, kind="ExternalInput").ap()
    w1d = nc.dram_tensor("w1", [E, E], mdt, kind="ExternalInput").ap()
    w2d = nc.dram_tensor("w2t", [E, E], mdt, kind="ExternalInput").ap()
    bdd = nc.dram_tensor("band", [256 + 2 * R, 256], mdt,
                         kind="ExternalInput").ap()
    od = nc.dram_tensor("out", [ROWS, E], f32, kind="ExternalOutput").ap()

    with tile.TileContext(nc) as tc, ExitStack() as ctx:
        xp = ctx.enter_context(tc.tile_pool(name="xp", bufs=N_XT))
        w1p = ctx.enter_context(tc.tile_pool(name="w1p", bufs=4))
        w2tp = ctx.enter_context(tc.tile_pool(name="w2tp", bufs=4))
        wctp = ctx.enter_context(tc.tile_pool(name="wctp", bufs=4))
        bp = ctx.enter_context(tc.tile_pool(name="bp", bufs=1))
        cvp = ctx.enter_context(tc.tile_pool(name="cvp", bufs=8))
        outp = ctx.enter_context(tc.tile_pool(name="outp", bufs=3))
        psA = ctx.enter_context(tc.tile_pool(name="psA", bufs=3, space="PSUM"))
        psB = ctx.enter_context(tc.tile_pool(name="psB", bufs=3, space="PSUM"))

        # ------------------------------------------------ load inputs
        # DMA issue order IS sync-engine execution order: small weight/band
        # loads first so the one-time WcT matmuls and the first conv block
        # start within ~4us; the bulk x stream follows and stays ahead of
        # the conv consumer.  Output DMAs go out on the scalar engine's
        # separate HWDGE ring (see below) so they don't queue behind x loads.
        b0 = bp.tile([128, 256], mdt, tag="b0")
        nc.sync.dma_start(out=b0[:], in_=bdd[0:128, :])
        b1 = bp.tile([128, 256], mdt, tag="b1")
        nc.sync.dma_start(out=b1[:], in_=bdd[128:256, :])
        b2 = bp.tile([2 * R, 256], mdt, tag="b2")
        nc.sync.dma_start(out=b2[:], in_=bdd[256:256 + 2 * R, :])

        w1t = []
        w2T = []
        for i in range(4):
            t1 = w1p.tile([128, E], mdt, tag="w1", name=f"w1_{i}")
            nc.sync.dma_start(out=t1[:], in_=w1d[128 * i:128 * i + 128, :])
            w1t.append(t1)
            t2 = w2tp.tile([128, E], mdt, tag="w2T", name=f"w2T_{i}")
            nc.sync.dma_start(out=t2[:], in_=w2d[128 * i:128 * i + 128, :])
            w2T.append(t2)

        xt = []
        for u in range(N_XT):
            t = xp.tile([128, E], mdt, tag="xt", name=f"xt{u}")
            nc.sync.dma_start(out=t[:], in_=xd[128 * u:128 * u + 128, :])
            xt.append(t)

        # ------------------------------------------------ WcT[e, f] = sum_m W1[m,e] W2T[m,f]
        wcT = [wctp.tile([128, E], mdt, tag="wcT", name=f"wcT_{i}") for i in range(4)]
        for ei in range(4):
            pw = psB.tile([128, E], f32, tag="psB", name="psB_t")
            for mi in range(4):
                nc.tensor.matmul(
                    pw[:],
                    w1t[mi][:, 128 * ei:128 * ei + 128],
                    w2T[mi][:],
                    start=(mi == 0),
                    stop=(mi == 3),
                )
            nc.vector.tensor_copy(wcT[ei][:], pw[:])

        # ------------------------------------------------ main loop
        for t in range(N_BLK):
            # conv block t: xcT[e, 256t + j] = sum_k x[k, e] band[k - 256t, j]
            xcT = []
            for ei in range(4):
                pc = psA.tile([128, 256], f32, tag="psA", name="psA_t")
                es = slice(128 * ei, 128 * ei + 128)
                nc.tensor.matmul(pc[:], xt[2 * t][:, es], b0[:],
                                 start=True, stop=False)
                nc.tensor.matmul(pc[:], xt[2 * t + 1][:, es], b1[:],
                                 start=False, stop=False)
                nc.tensor.matmul(pc[:], xt[2 * t + 2][0:2 * R, es], b2[:],
                                 start=False, stop=True)
                ct = cvp.tile([128, 256], mdt, tag="xcT", name=f"xcT_{t}_{ei}")
                nc.vector.tensor_copy(ct[:], pc[:])
                xcT.append(ct)

            for q in range(2):           # two 128-row output tiles per block
                r = 2 * t + q
                po = psB.tile([128, E], f32, tag="psB", name="psB_t")
                ss = slice(128 * q, 128 * q + 128)
                for ei in range(4):
                    nc.tensor.matmul(po[:], xcT[ei][:, ss], wcT[ei][:],
                                     start=(ei == 0), stop=(ei == 3))
                ot = outp.tile([128, E], f32, tag="ot", name=f"ot{r}")
                nc.vector.tensor_copy(ot[:], po[:])
                nc.scalar.dma_start(out=od[128 * r:128 * r + 128, :], in_=ot[:])

    nc.compile()
    return nc


def _get_nc(dtype_flag: str):
    with _lock:
        if dtype_flag not in _cache:
            _cache[dtype_flag] = _build(dtype_flag)
        return _cache[dtype_flag]


def _np_dtype(dtype_flag: str):
    if dtype_flag == "bf16":
        import ml_dtypes
        return ml_dtypes.bfloat16
    return np.float32


def kernel(inputs: np.ndarray, input_weights: np.ndarray,
           output_weight: np.ndarray) -> np.ndarray:
    x = np.ascontiguousarray(np.asarray(inputs, dtype=np.float32))
    w1 = np.asarray(input_weights, dtype=np.float32)
    w2 = np.asarray(output_weight, dtype=np.float32)
    assert x.shape == (B, S, E) and w1.shape == (E, E) and w2.shape == (E, E)

    nc = _get_nc(DTYPE)
    ndt = _np_dtype(DTYPE)

    w1s = np.ascontiguousarray(w1.astype(ndt))
    w2s = np.ascontiguousarray(w2.T.astype(ndt))   # ship W2 pre-transposed
    band = np.ascontiguousarray(_band_matrix().astype(ndt))

    halves = S // 2                  # rows per core within a batch
    in_maps = []
    for c in range(N_CORES):
        b, half = divmod(c, 2)
        s0 = half * halves
        s1 = s0 + halves
        xc = np.zeros((XPAD, E), dtype=ndt)
        xc[R:R + ROWS] = x[b, s0:s1].astype(ndt)
        if s0 > 0:
            xc[0:R] = x[b, s0 - R:s0].astype(ndt)
        if s1 < S:
            xc[R + ROWS:R + ROWS + R] = x[b, s1:s1 + R].astype(ndt)
        in_maps.append({"x": xc, "w1": w1s, "w2t": w2s, "band": band})

    res = run_bass_kernel_spmd(nc, in_maps, core_ids=list(range(N_CORES)))

    out = np.empty((B, S, E), dtype=np.float32)
    for c in range(N_CORES):
        b, half = divmod(c, 2)
        s0 = half * halves
        out[b, s0:s0 + halves] = res.results[c]["out"]
    return out


# revision 6
# speedup vs baseline: 1.2880x; 1.1648x over previous
"""Trainium2 Bass kernel for nn_NewAttention_55344948576827.

Math: reference computes
    v   = x @ W1.T                      (x: [B,S,E], W1: [E,E])
    att = w_pos @ v  (per head)         (w_pos[q,k] = c*exp(-0.5*(k-q)^2), [S,S])
    out = att @ W2.T
The positional weights are data independent, identical for every head and
channel, and decay below fp32 noise for |k-q| > 8 — so the [S,S] matmul is a
17-tap 1D convolution along S.  The conv acts on the S axis only and both
projections act on the E axis only, so they commute:
    out = conv_S(x) @ (W2 @ W1).T
One fused weight matrix, one conv.

Device mapping (per core):
  - conv_S as PE matmuls against a constant banded [272, 256] matrix; with x
    tiles as the stationary operand this *also* transposes x (e lands on
    partitions), exactly what the main matmul needs.
  - WcT = W1.T @ W2.T computed on-device once per core (4 accumulating
    matmuls; W2 is shipped pre-transposed from the host — a zero-FLOP
    relayout, like the halo prep).
  - main: out[s,f] = sum_e xcT[e,s] * WcT[e,f], 4 accumulating matmuls per
    128-row output tile.

Sharding: B*S = 16384 rows split 8 ways -> 2048 rows/core (half a batch, so
the conv never crosses a core's slice except through an 8-row halo baked into
the shipped input).  No collectives.

Layout/perf notes:
  - All inputs are shipped in [128, n, free] partition-major layout so each
    load is one large contiguous-per-partition DMA (DMA issue on the sync
    sequencer costs ~0.6us per instruction — fewer, bigger DMAs).
  - Conv psum uses one [128,512] bank per e-chunk PAIR (two 256-col matmul
    groups) so PSUM->SBUF traffic is 16 copies instead of 32.
  - Output copies alternate DVE / ScalarE; output DMAs go on the scalar
    engine's HWDGE ring, input DMAs on the sync ring.
"""

import os
import threading
from contextlib import ExitStack

import numpy as np

import concourse.bass as bass
import concourse.tile as tile
from concourse import bacc, mybir
from concourse.bass_utils import run_bass_kernel_spmd

# ---------------------------------------------------------------- constants
B, S, E = 4, 4096, 512
N_CORES = 8
ROWS = (B * S) // N_CORES          # 2048 rows per core
R = 8                              # gaussian band radius (17 taps)
XPAD = 2176                        # 2048 + 2*R halo, padded to 17 tiles of 128
N_XT = XPAD // 128                 # 17 x tiles
X_GROUPS = (3, 4, 4, 6)            # x tiles per DMA batch (first small ->
                                   # conv block 0 unblocks early)
N_BLK = ROWS // 256                # 8 conv blocks of 256 output rows

# matmul dtype: "bf16" (inputs quantized host-side, ~4e-3 rel err) or
# "f32r" (full fp32 data, relaxed-precision PE mode, ~2.5e-4 rel err, slower)
DTYPE = os.environ.get("KERNEL_DTYPE", "bf16")

_lock = threading.Lock()
_cache = {}


def _band_matrix() -> np.ndarray:
    """Constant [272, 256] band matrix: band[k, j] = g(k - j - R)."""
    coef = np.float32(1.0 / np.sqrt(2.0 * np.pi))
    band = np.zeros((256 + 2 * R, 256), dtype=np.float64)
    k = np.arange(256 + 2 * R)[:, None]
    j = np.arange(256)[None, :]
    d = k - j - R
    mask = np.abs(d) <= R
    band[mask] = (coef * np.exp(-0.5 * d.astype(np.float64) ** 2))[mask]
    return band.astype(np.float32)


def _build(dtype_flag: str):
    mdt = {"f32r": mybir.dt.float32r, "bf16": mybir.dt.bfloat16}[dtype_flag]
    f32 = mybir.dt.float32

    nc = bacc.Bacc("TRN2", target_bir_lowering=False, debug=False,
                   num_devices=N_CORES)

    # partition-major input layouts: [128, ntiles, free]
    xd = nc.dram_tensor("x", [128, N_XT, E], mdt, kind="ExternalInput").ap()
    w1d = nc.dram_tensor("w1", [128, 4, E], mdt, kind="ExternalInput").ap()
    w2d = nc.dram_tensor("w2t", [128, 4, E], mdt, kind="ExternalInput").ap()
    b01d = nc.dram_tensor("band01", [128, 2, 256], mdt,
                          kind="ExternalInput").ap()
    b2d = nc.dram_tensor("band2", [2 * R, 256], mdt, kind="ExternalInput").ap()
    od = nc.dram_tensor("out", [ROWS, E], f32, kind="ExternalOutput").ap()

    with tile.TileContext(nc) as tc, ExitStack() as ctx:
        xp = ctx.enter_context(tc.tile_pool(name="xp", bufs=1))
        wp = ctx.enter_context(tc.tile_pool(name="wp", bufs=1))
        wctp = ctx.enter_context(tc.tile_pool(name="wctp", bufs=4))
        cvp = ctx.enter_context(tc.tile_pool(name="cvp", bufs=4))
        outp = ctx.enter_context(tc.tile_pool(name="outp", bufs=4))
        psA = ctx.enter_context(tc.tile_pool(name="psA", bufs=3, space="PSUM"))
        psB = ctx.enter_context(tc.tile_pool(name="psB", bufs=3, space="PSUM"))

        # ------------------------------------------------ input DMAs
        # issue order = sync-engine execution order: small weight/band loads
        # first so WcT + conv block 0 start early; x batches follow.
        b01 = wp.tile([128, 2, 256], mdt, tag="b01")
        nc.sync.dma_start(out=b01[:], in_=b01d[:])
        b2 = wp.tile([2 * R, 256], mdt, tag="b2")
        nc.sync.dma_start(out=b2[:], in_=b2d[:])
        w1b = wp.tile([128, 4, E], mdt, tag="w1b")
        nc.sync.dma_start(out=w1b[:], in_=w1d[:])
        w2b = wp.tile([128, 4, E], mdt, tag="w2b")
        nc.sync.dma_start(out=w2b[:], in_=w2d[:])

        xg = []
        off = 0
        for gi, gsz in enumerate(X_GROUPS):
            g = xp.tile([128, gsz, E], mdt, tag=f"xg{gi}", name=f"xg{gi}")
            nc.sync.dma_start(out=g[:], in_=xd[:, off:off + gsz, :])
            xg.append((off, gsz, g))
            off += gsz

        def xt(u):
            for off_, gsz_, g_ in xg:
                if off_ <= u < off_ + gsz_:
                    return g_[:, u - off_, :]
            raise IndexError(u)

        bnd = [b01[:, 0, :], b01[:, 1, :], b2[:]]

        # --------------------------- WcT[e,f] = sum_m W1[m,e] W2T[m,f]
        wcT = [wctp.tile([128, E], mdt, tag="wcT", name=f"wcT_{i}")
               for i in range(4)]
        for ei in range(4):
            pw = psB.tile([128, E], f32, tag="psB", name="psB_t")
            for mi in range(4):
                nc.tensor.matmul(
                    pw[:],
                    w1b[:, mi, 128 * ei:128 * ei + 128],
                    w2b[:, mi, :],
                    start=(mi == 0),
                    stop=(mi == 3),
                )
            nc.vector.tensor_copy(wcT[ei][:], pw[:])

        # ------------------------------------------------ main loop
        for t in range(N_BLK):
            # conv block t: xcT[e, 256t + j] = sum_k x[k, e] band[k - 256t, j]
            # two e-chunks share one [128, 512] psum bank (cols 0:256/256:512)
            xcT = []
            for pi in range(2):
                pc = psA.tile([128, 512], f32, tag="psA", name="psA_t")
                for sub in range(2):
                    ei = 2 * pi + sub
                    es = slice(128 * ei, 128 * ei + 128)
                    dst = pc[:, 256 * sub:256 * sub + 256]
                    nc.tensor.matmul(dst, xt(2 * t)[:, es], bnd[0],
                                     start=True, stop=False)
                    nc.tensor.matmul(dst, xt(2 * t + 1)[:, es], bnd[1],
                                     start=False, stop=False)
                    nc.tensor.matmul(dst, xt(2 * t + 2)[0:2 * R, es], bnd[2],
                                     start=False, stop=True)
                ct = cvp.tile([128, 512], mdt, tag="xcT", name=f"xcT_{t}_{pi}")
                nc.vector.tensor_copy(ct[:], pc[:])
                xcT.append(ct)

            for q in range(2):           # two 128-row output tiles per block
                r = 2 * t + q
                po = psB.tile([128, E], f32, tag="psB", name="psB_t")
                for ei in range(4):
                    pi, sub = divmod(ei, 2)
                    ss = slice(256 * sub + 128 * q, 256 * sub + 128 * q + 128)
                    nc.tensor.matmul(po[:], xcT[pi][:, ss], wcT[ei][:],
                                     start=(ei == 0), stop=(ei == 3))
                ot = outp.tile([128, E], f32, tag="ot", name=f"ot{r}")
                if q == 0:
                    nc.vector.tensor_copy(ot[:], po[:])
                else:
                    nc.scalar.copy(ot[:], po[:])
                nc.scalar.dma_start(out=od[128 * r:128 * r + 128, :],
                                    in_=ot[:])

    nc.compile()
    return nc


def _get_nc(dtype_flag: str):
    with _lock:
        if dtype_flag not in _cache:
            _cache[dtype_flag] = _build(dtype_flag)
        return _cache[dtype_flag]


def _np_dtype(dtype_flag: str):
    if dtype_flag == "bf16":
        import ml_dtypes
        return ml_dtypes.bfloat16
    return np.float32


def _part_major(a: np.ndarray) -> np.ndarray:
    """[n*128, free] -> [128, n, free] (partition-major DMA layout)."""
    n = a.shape[0] // 128
    return np.ascontiguousarray(
        a.reshape(n, 128, a.shape[1]).transpose(1, 0, 2))


def make_in_maps(x: np.ndarray, w1: np.ndarray, w2: np.ndarray,
                 dtype_flag: str):
    ndt = _np_dtype(dtype_flag)
    band = _band_matrix()
    w1s = _part_major(w1.astype(ndt))
    w2s = _part_major(np.ascontiguousarray(w2.T).astype(ndt))
    b01 = _part_major(band[0:256].astype(ndt))
    b2 = np.ascontiguousarray(band[256:256 + 2 * R].astype(ndt))

    halves = S // 2
    in_maps = []
    for c in range(N_CORES):
        b, half = divmod(c, 2)
        s0 = half * halves
        s1 = s0 + halves
        xc = np.zeros((XPAD, E), dtype=ndt)
        xc[R:R + ROWS] = x[b, s0:s1].astype(ndt)
        if s0 > 0:
            xc[0:R] = x[b, s0 - R:s0].astype(ndt)
        if s1 < S:
            xc[R + ROWS:R + ROWS + R] = x[b, s1:s1 + R].astype(ndt)
        in_maps.append({"x": _part_major(xc), "w1": w1s, "w2t": w2s,
                        "band01": b01, "band2": b2})
    return in_maps


def kernel(inputs: np.ndarray, input_weights: np.ndarray,
           output_weight: np.ndarray) -> np.ndarray:
    x = np.ascontiguousarray(np.asarray(inputs, dtype=np.float32))
    w1 = np.asarray(input_weights, dtype=np.float32)
    w2 = np.asarray(output_weight, dtype=np.float32)
    assert x.shape == (B, S, E) and w1.shape == (E, E) and w2.shape == (E, E)

    nc = _get_nc(DTYPE)
    in_maps = make_in_maps(x, w1, w2, DTYPE)
    res = run_bass_kernel_spmd(nc, in_maps, core_ids=list(range(N_CORES)))

    halves = S // 2
    out = np.empty((B, S, E), dtype=np.float32)
    for c in range(N_CORES):
        b, half = divmod(c, 2)
        s0 = half * halves
        out[b, s0:s0 + halves] = res.results[c]["out"]
    return out


# revision 7
# speedup vs baseline: 1.3158x; 1.0216x over previous
"""Trainium2 Bass kernel for nn_NewAttention_55344948576827.

Math: reference computes
    v   = x @ W1.T                      (x: [B,S,E], W1: [E,E])
    att = w_pos @ v  (per head)         (w_pos[q,k] = c*exp(-0.5*(k-q)^2), [S,S])
    out = att @ W2.T
The positional weights are data independent, identical for every head and
channel, and decay below fp32 noise for |k-q| > 8 — so the [S,S] matmul is a
17-tap 1D convolution along S.  The conv acts on the S axis only and both
projections act on the E axis only, so they commute:
    out = conv_S(x) @ (W2 @ W1).T
One fused weight matrix, one conv.

Device mapping (per core):
  - conv_S as PE matmuls against a constant banded [272, 256] matrix; with x
    tiles as the stationary operand this *also* transposes x (e lands on
    partitions), exactly what the main matmul needs.
  - WcT = W1.T @ W2.T computed on-device once per core (4 accumulating
    matmuls; W2 is shipped pre-transposed from the host — a zero-FLOP
    relayout, like the halo prep).
  - main: out[s,f] = sum_e xcT[e,s] * WcT[e,f], 4 accumulating matmuls per
    128-row output tile.

Sharding: B*S = 16384 rows split 8 ways -> 2048 rows/core (half a batch, so
the conv never crosses a core's slice except through an 8-row halo baked into
the shipped input).  No collectives.

Layout/perf notes:
  - All inputs are shipped in [128, n, free] partition-major layout so each
    load is one large contiguous-per-partition DMA (DMA issue on the sync
    sequencer costs ~0.6us per instruction — fewer, bigger DMAs).
  - Conv psum uses one [128,512] bank per e-chunk PAIR (two 256-col matmul
    groups) so PSUM->SBUF traffic is 16 copies instead of 32.
  - Output copies alternate DVE / ScalarE; output DMAs go on the scalar
    engine's HWDGE ring, input DMAs on the sync ring.
"""

import os
import threading
from contextlib import ExitStack

import numpy as np

import concourse.bass as bass
import concourse.tile as tile
from concourse import bacc, mybir
from concourse.bass_utils import run_bass_kernel_spmd

# ---------------------------------------------------------------- constants
B, S, E = 4, 4096, 512
N_CORES = 8
ROWS = (B * S) // N_CORES          # 2048 rows per core
R = 8                              # gaussian band radius (17 taps)
XPAD = 2176                        # 2048 + 2*R halo, padded to 17 tiles of 128
N_XT = XPAD // 128                 # 17 x tiles
X_GROUPS = (3, 4, 4, 6)            # x tiles per DMA batch (first small ->
                                   # conv block 0 unblocks early)
N_BLK = ROWS // 256                # 8 conv blocks of 256 output rows

# matmul dtype: "bf16" (inputs quantized host-side, ~4e-3 rel err) or
# "f32r" (full fp32 data, relaxed-precision PE mode, ~2.5e-4 rel err, slower)
DTYPE = os.environ.get("KERNEL_DTYPE", "bf16")

_lock = threading.Lock()
_cache = {}


def _band_matrix() -> np.ndarray:
    """Constant [272, 256] band matrix: band[k, j] = g(k - j - R)."""
    coef = np.float32(1.0 / np.sqrt(2.0 * np.pi))
    band = np.zeros((256 + 2 * R, 256), dtype=np.float64)
    k = np.arange(256 + 2 * R)[:, None]
    j = np.arange(256)[None, :]
    d = k - j - R
    mask = np.abs(d) <= R
    band[mask] = (coef * np.exp(-0.5 * d.astype(np.float64) ** 2))[mask]
    return band.astype(np.float32)


def _build(dtype_flag: str):
    mdt = {"f32r": mybir.dt.float32r, "bf16": mybir.dt.bfloat16}[dtype_flag]
    f32 = mybir.dt.float32

    nc = bacc.Bacc("TRN2", target_bir_lowering=False, debug=False,
                   num_devices=N_CORES)

    # partition-major input layouts: [128, ntiles, free]
    xd = nc.dram_tensor("x", [128, N_XT, E], mdt, kind="ExternalInput").ap()
    w1d = nc.dram_tensor("w1", [128, 4, E], mdt, kind="ExternalInput").ap()
    w2d = nc.dram_tensor("w2t", [128, 4, E], mdt, kind="ExternalInput").ap()
    b01d = nc.dram_tensor("band01", [128, 2, 256], mdt,
                          kind="ExternalInput").ap()
    b2d = nc.dram_tensor("band2", [2 * R, 256], mdt, kind="ExternalInput").ap()
    od = nc.dram_tensor("out", [ROWS, E], f32, kind="ExternalOutput").ap()

    with tile.TileContext(nc) as tc, ExitStack() as ctx:
        xp = ctx.enter_context(tc.tile_pool(name="xp", bufs=1))
        wp = ctx.enter_context(tc.tile_pool(name="wp", bufs=1))
        wctp = ctx.enter_context(tc.tile_pool(name="wctp", bufs=4))
        cvp = ctx.enter_context(tc.tile_pool(name="cvp", bufs=4))
        outp = ctx.enter_context(tc.tile_pool(name="outp", bufs=4))
        psA = ctx.enter_context(tc.tile_pool(name="psA", bufs=4, space="PSUM"))
        psB = ctx.enter_context(tc.tile_pool(name="psB", bufs=4, space="PSUM"))

        # ------------------------------------------------ input DMAs
        # issue order = sync-engine execution order: small weight/band loads
        # first so WcT + conv block 0 start early; x batches follow.
        b01 = wp.tile([128, 2, 256], mdt, tag="b01")
        nc.sync.dma_start(out=b01[:], in_=b01d[:])
        b2 = wp.tile([2 * R, 256], mdt, tag="b2")
        nc.sync.dma_start(out=b2[:], in_=b2d[:])

        xg = []

        def _load_xg(gi, off, gsz):
            g = xp.tile([128, gsz, E], mdt, tag=f"xg{gi}", name=f"xg{gi}")
            nc.sync.dma_start(out=g[:], in_=xd[:, off:off + gsz, :])
            xg.append((off, gsz, g))

        _load_xg(0, 0, X_GROUPS[0])

        w1b = wp.tile([128, 4, E], mdt, tag="w1b")
        nc.sync.dma_start(out=w1b[:], in_=w1d[:])
        w2b = wp.tile([128, 4, E], mdt, tag="w2b")
        nc.sync.dma_start(out=w2b[:], in_=w2d[:])

        off = X_GROUPS[0]
        for gi, gsz in enumerate(X_GROUPS[1:], start=1):
            _load_xg(gi, off, gsz)
            off += gsz

        def xt(u):
            for off_, gsz_, g_ in xg:
                if off_ <= u < off_ + gsz_:
                    return g_[:, u - off_, :]
            raise IndexError(u)

        bnd = [b01[:, 0, :], b01[:, 1, :], b2[:]]

        def conv_block(t):
            # conv block t: xcT[e, 256t + j] = sum_k x[k, e] band[k - 256t, j]
            # two e-chunks share one [128, 512] psum bank (cols 0:256/256:512)
            xcT = []
            for pi in range(2):
                pc = psA.tile([128, 512], f32, tag="psA", name="psA_t")
                for sub in range(2):
                    ei = 2 * pi + sub
                    es = slice(128 * ei, 128 * ei + 128)
                    dst = pc[:, 256 * sub:256 * sub + 256]
                    nc.tensor.matmul(dst, xt(2 * t)[:, es], bnd[0],
                                     start=True, stop=False)
                    nc.tensor.matmul(dst, xt(2 * t + 1)[:, es], bnd[1],
                                     start=False, stop=False)
                    nc.tensor.matmul(dst, xt(2 * t + 2)[0:2 * R, es], bnd[2],
                                     start=False, stop=True)
                ct = cvp.tile([128, 512], mdt, tag="xcT", name=f"xcT_{t}_{pi}")
                nc.vector.tensor_copy(ct[:], pc[:])
                xcT.append(ct)
            return xcT

        def main_block(t, xcT, wcT):
            for q in range(2):           # two 128-row output tiles per block
                r = 2 * t + q
                po = psB.tile([128, E], f32, tag="psB", name="psB_t")
                for ei in range(4):
                    pi, sub = divmod(ei, 2)
                    ss = slice(256 * sub + 128 * q, 256 * sub + 128 * q + 128)
                    nc.tensor.matmul(po[:], xcT[pi][:, ss], wcT[ei][:],
                                     start=(ei == 0), stop=(ei == 3))
                ot = outp.tile([128, E], f32, tag="ot", name=f"ot{r}")
                if q == 0:
                    nc.vector.tensor_copy(ot[:], po[:])
                else:
                    nc.scalar.copy(ot[:], po[:])
                nc.scalar.dma_start(out=od[128 * r:128 * r + 128, :],
                                    in_=ot[:])

        # PE program order tracks DMA arrival: conv block 0 starts on the
        # early xg0+band loads, WcT runs while xg1 streams, then the steady
        # conv/main pipeline.
        xcT0 = conv_block(0)

        # --------------------------- WcT[e,f] = sum_m W1[m,e] W2T[m,f]
        wcT = [wctp.tile([128, E], mdt, tag="wcT", name=f"wcT_{i}")
               for i in range(4)]
        for ei in range(4):
            pw = psB.tile([128, E], f32, tag="psB", name="psB_t")
            for mi in range(4):
                nc.tensor.matmul(
                    pw[:],
                    w1b[:, mi, 128 * ei:128 * ei + 128],
                    w2b[:, mi, :],
                    start=(mi == 0),
                    stop=(mi == 3),
                )
            nc.vector.tensor_copy(wcT[ei][:], pw[:])

        main_block(0, xcT0, wcT)
        for t in range(1, N_BLK):
            xcT = conv_block(t)
            main_block(t, xcT, wcT)

    nc.compile()
    return nc


def _get_nc(dtype_flag: str):
    with _lock:
        if dtype_flag not in _cache:
            _cache[dtype_flag] = _build(dtype_flag)
        return _cache[dtype_flag]


def _np_dtype(dtype_flag: str):
    if dtype_flag == "bf16":
        import ml_dtypes
        return ml_dtypes.bfloat16
    return np.float32


def _part_major(a: np.ndarray) -> np.ndarray:
    """[n*128, free] -> [128, n, free] (partition-major DMA layout)."""
    n = a.shape[0] // 128
    return np.ascontiguousarray(
        a.reshape(n, 128, a.shape[1]).transpose(1, 0, 2))


def make_in_maps(x: np.ndarray, w1: np.ndarray, w2: np.ndarray,
                 dtype_flag: str):
    ndt = _np_dtype(dtype_flag)
    band = _band_matrix()
    w1s = _part_major(w1.astype(ndt))
    w2s = _part_major(np.ascontiguousarray(w2.T).astype(ndt))
    b01 = _part_major(band[0:256].astype(ndt))
    b2 = np.ascontiguousarray(band[256:256 + 2 * R].astype(ndt))

    halves = S // 2
    in_maps = []
    for c in range(N_CORES):
        b, half = divmod(c, 2)
        s0 = half * halves
        s1 = s0 + halves
        xc = np.zeros((XPAD, E), dtype=ndt)
        xc[R:R + ROWS] = x[b, s0:s1].astype(ndt)
        if s0 > 0:
            xc[0:R] = x[b, s0 - R:s0].astype(ndt)
        if s1 < S:
            xc[R + ROWS:R + ROWS + R] = x[b, s1:s1 + R].astype(ndt)
        in_maps.append({"x": _part_major(xc), "w1": w1s, "w2t": w2s,
                        "band01": b01, "band2": b2})
    return in_maps


def kernel(inputs: np.ndarray, input_weights: np.ndarray,
           output_weight: np.ndarray) -> np.ndarray:
    x = np.ascontiguousarray(np.asarray(inputs, dtype=np.float32))
    w1 = np.asarray(input_weights, dtype=np.float32)
    w2 = np.asarray(output_weight, dtype=np.float32)
    assert x.shape == (B, S, E) and w1.shape == (E, E) and w2.shape == (E, E)

    nc = _get_nc(DTYPE)
    in_maps = make_in_maps(x, w1, w2, DTYPE)
    res = run_bass_kernel_spmd(nc, in_maps, core_ids=list(range(N_CORES)))

    halves = S // 2
    out = np.empty((B, S, E), dtype=np.float32)
    for c in range(N_CORES):
        b, half = divmod(c, 2)
        s0 = half * halves
        out[b, s0:s0 + halves] = res.results[c]["out"]
    return out


# revision 8
# speedup vs baseline: 1.4153x; 1.0756x over previous
"""Trainium2 Bass kernel for nn_NewAttention_55344948576827.

Math: reference computes
    v   = x @ W1.T                      (x: [B,S,E], W1: [E,E])
    att = w_pos @ v  (per head)         (w_pos[q,k] = c*exp(-0.5*(k-q)^2), [S,S])
    out = att @ W2.T
The positional weights are data independent, identical for every head and
channel, and decay below fp32 noise for |k-q| > 8 — so the [S,S] matmul is a
17-tap 1D convolution along S.  The conv acts on the S axis only and both
projections act on the E axis only, so they commute:
    out = conv_S(x) @ (W2 @ W1).T
One fused weight matrix, one conv.

Device mapping (per core):
  - conv_S as PE matmuls against a constant banded [272, 256] matrix; with x
    tiles as the stationary operand this *also* transposes x (e lands on
    partitions), exactly what the main matmul needs.
  - WcT = W1.T @ W2.T computed on-device once per core (4 accumulating
    matmuls; W2 is shipped pre-transposed from the host — a zero-FLOP
    relayout, like the halo prep).
  - main: out[s,f] = sum_e xcT[e,s] * WcT[e,f], 4 accumulating matmuls per
    128-row output tile.

Sharding: B*S = 16384 rows split 8 ways -> 2048 rows/core (half a batch, so
the conv never crosses a core's slice except through an 8-row halo baked into
the shipped input).  No collectives.

Layout/perf notes:
  - All inputs are shipped in [128, n, free] partition-major layout so each
    load is one large contiguous-per-partition DMA (DMA issue on the sync
    sequencer costs ~0.6us per instruction — fewer, bigger DMAs).
  - Conv psum uses one [128,512] bank per e-chunk PAIR (two 256-col matmul
    groups) so PSUM->SBUF traffic is 16 copies instead of 32.
  - Output copies alternate DVE / ScalarE; output DMAs go on the scalar
    engine's HWDGE ring, input DMAs on the sync ring.
"""

import os
import threading
from contextlib import ExitStack

import numpy as np

import concourse.bass as bass
import concourse.tile as tile
from concourse import bacc, mybir
from concourse.bass_utils import run_bass_kernel_spmd

# ---------------------------------------------------------------- constants
B, S, E = 4, 4096, 512
N_CORES = 8
ROWS = (B * S) // N_CORES          # 2048 rows per core
R = 8                              # gaussian band radius (17 taps)
XPAD = 2176                        # 2048 + 2*R halo, padded to 17 tiles of 128
N_XT = XPAD // 128                 # 17 x tiles
X_GROUPS = (3, 4, 4, 6)            # x tiles per DMA batch (first small ->
                                   # conv block 0 unblocks early)
N_BLK = ROWS // 256                # 8 conv blocks of 256 output rows

# matmul dtype: "bf16" (inputs quantized host-side, ~4e-3 rel err) or
# "f32r" (full fp32 data, relaxed-precision PE mode, ~2.5e-4 rel err, slower)
DTYPE = os.environ.get("KERNEL_DTYPE", "bf16")

_lock = threading.Lock()
_cache = {}


def _band_matrix() -> np.ndarray:
    """Constant [272, 256] band matrix: band[k, j] = g(k - j - R)."""
    coef = np.float32(1.0 / np.sqrt(2.0 * np.pi))
    band = np.zeros((256 + 2 * R, 256), dtype=np.float64)
    k = np.arange(256 + 2 * R)[:, None]
    j = np.arange(256)[None, :]
    d = k - j - R
    mask = np.abs(d) <= R
    band[mask] = (coef * np.exp(-0.5 * d.astype(np.float64) ** 2))[mask]
    return band.astype(np.float32)


def _build(dtype_flag: str):
    mdt = {"f32r": mybir.dt.float32r, "bf16": mybir.dt.bfloat16}[dtype_flag]
    f32 = mybir.dt.float32

    nc = bacc.Bacc("TRN2", target_bir_lowering=False, debug=False,
                   num_devices=N_CORES)

    # partition-major input layouts: [128, ntiles, free]
    xd = nc.dram_tensor("x", [128, N_XT, E], mdt, kind="ExternalInput").ap()
    w1d = nc.dram_tensor("w1", [128, 4, E], mdt, kind="ExternalInput").ap()
    w2d = nc.dram_tensor("w2t", [128, 4, E], mdt, kind="ExternalInput").ap()
    b01d = nc.dram_tensor("band01", [128, 2, 256], mdt,
                          kind="ExternalInput").ap()
    b2d = nc.dram_tensor("band2", [2 * R, 256], mdt, kind="ExternalInput").ap()
    od = nc.dram_tensor("out", [ROWS, E], f32, kind="ExternalOutput").ap()

    with tile.TileContext(nc) as tc, ExitStack() as ctx:
        xp = ctx.enter_context(tc.tile_pool(name="xp", bufs=1))
        wp = ctx.enter_context(tc.tile_pool(name="wp", bufs=1))
        wctp = ctx.enter_context(tc.tile_pool(name="wctp", bufs=4))
        cvp = ctx.enter_context(tc.tile_pool(name="cvp", bufs=4))
        outp = ctx.enter_context(tc.tile_pool(name="outp", bufs=4))
        psA = ctx.enter_context(tc.tile_pool(name="psA", bufs=4, space="PSUM"))
        psB = ctx.enter_context(tc.tile_pool(name="psB", bufs=4, space="PSUM"))

        # ------------------------------------------------ input DMAs
        # issue order = sync-engine execution order: small weight/band loads
        # first so WcT + conv block 0 start early; x batches follow.
        b01 = wp.tile([128, 2, 256], mdt, tag="b01")
        nc.sync.dma_start(out=b01[:], in_=b01d[:])
        b2 = wp.tile([2 * R, 256], mdt, tag="b2")
        nc.sync.dma_start(out=b2[:], in_=b2d[:])

        xg = []

        def _load_xg(gi, off, gsz):
            g = xp.tile([128, gsz, E], mdt, tag=f"xg{gi}", name=f"xg{gi}")
            nc.sync.dma_start(out=g[:], in_=xd[:, off:off + gsz, :])
            xg.append((off, gsz, g))

        _load_xg(0, 0, X_GROUPS[0])

        w1b = wp.tile([128, 4, E], mdt, tag="w1b")
        nc.scalar.dma_start(out=w1b[:], in_=w1d[:])
        w2b = wp.tile([128, 4, E], mdt, tag="w2b")
        nc.scalar.dma_start(out=w2b[:], in_=w2d[:])

        off = X_GROUPS[0]
        for gi, gsz in enumerate(X_GROUPS[1:], start=1):
            _load_xg(gi, off, gsz)
            off += gsz

        def xt(u):
            for off_, gsz_, g_ in xg:
                if off_ <= u < off_ + gsz_:
                    return g_[:, u - off_, :]
            raise IndexError(u)

        bnd = [b01[:, 0, :], b01[:, 1, :], b2[:]]

        def conv_block(t):
            # conv block t: xcT[e, 256t + j] = sum_k x[k, e] band[k - 256t, j]
            # two e-chunks share one [128, 512] psum bank (cols 0:256/256:512)
            xcT = []
            for pi in range(2):
                pc = psA.tile([128, 512], f32, tag="psA", name="psA_t")
                for sub in range(2):
                    ei = 2 * pi + sub
                    es = slice(128 * ei, 128 * ei + 128)
                    dst = pc[:, 256 * sub:256 * sub + 256]
                    # band chunk column support: B1 -> [0,256) (values in
                    # [112,256)), B0 -> [0,128), B2 -> [240,256).  Issue the
                    # full-width chunk first with start=True, then accumulate
                    # the narrow ones — 256+128+16 cols streamed vs 3*256.
                    nc.tensor.matmul(dst, xt(2 * t + 1)[:, es], bnd[1],
                                     start=True, stop=False)
                    nc.tensor.matmul(pc[:, 256 * sub:256 * sub + 128],
                                     xt(2 * t)[:, es], bnd[0][:, 0:128],
                                     start=False, stop=False)
                    nc.tensor.matmul(pc[:, 256 * sub + 240:256 * sub + 256],
                                     xt(2 * t + 2)[0:2 * R, es],
                                     bnd[2][:, 240:256],
                                     start=False, stop=True)
                ct = cvp.tile([128, 512], mdt, tag="xcT", name=f"xcT_{t}_{pi}")
                nc.vector.tensor_copy(ct[:], pc[:])
                xcT.append(ct)
            return xcT

        def main_block(t, xcT, wcT):
            for q in range(2):           # two 128-row output tiles per block
                r = 2 * t + q
                po = psB.tile([128, E], f32, tag="psB", name="psB_t")
                for ei in range(4):
                    pi, sub = divmod(ei, 2)
                    ss = slice(256 * sub + 128 * q, 256 * sub + 128 * q + 128)
                    nc.tensor.matmul(po[:], xcT[pi][:, ss], wcT[ei][:],
                                     start=(ei == 0), stop=(ei == 3))
                ot = outp.tile([128, E], f32, tag="ot", name=f"ot{r}")
                if q == 0:
                    nc.vector.tensor_copy(ot[:], po[:])
                else:
                    nc.scalar.copy(ot[:], po[:])
                eng = nc.scalar if q == 0 else nc.sync
                eng.dma_start(out=od[128 * r:128 * r + 128, :], in_=ot[:])

        # PE program order tracks DMA arrival: conv block 0 starts on the
        # early xg0+band loads, WcT runs while xg1 streams, then the steady
        # conv/main pipeline.
        xcT0 = conv_block(0)

        # --------------------------- WcT[e,f] = sum_m W1[m,e] W2T[m,f]
        wcT = [wctp.tile([128, E], mdt, tag="wcT", name=f"wcT_{i}")
               for i in range(4)]
        for ei in range(4):
            pw = psB.tile([128, E], f32, tag="psB", name="psB_t")
            for mi in range(4):
                nc.tensor.matmul(
                    pw[:],
                    w1b[:, mi, 128 * ei:128 * ei + 128],
                    w2b[:, mi, :],
                    start=(mi == 0),
                    stop=(mi == 3),
                )
            nc.vector.tensor_copy(wcT[ei][:], pw[:])

        main_block(0, xcT0, wcT)
        for t in range(1, N_BLK):
            xcT = conv_block(t)
            main_block(t, xcT, wcT)

    nc.compile()
    return nc


def _get_nc(dtype_flag: str):
    with _lock:
        if dtype_flag not in _cache:
            _cache[dtype_flag] = _build(dtype_flag)
        return _cache[dtype_flag]


def _np_dtype(dtype_flag: str):
    if dtype_flag == "bf16":
        import ml_dtypes
        return ml_dtypes.bfloat16
    return np.float32


def _part_major(a: np.ndarray) -> np.ndarray:
    """[n*128, free] -> [128, n, free] (partition-major DMA layout)."""
    n = a.shape[0] // 128
    return np.ascontiguousarray(
        a.reshape(n, 128, a.shape[1]).transpose(1, 0, 2))


def make_in_maps(x: np.ndarray, w1: np.ndarray, w2: np.ndarray,
                 dtype_flag: str):
    ndt = _np_dtype(dtype_flag)
    band = _band_matrix()
    w1s = _part_major(w1.astype(ndt))
    w2s = _part_major(np.ascontiguousarray(w2.T).astype(ndt))
    b01 = _part_major(band[0:256].astype(ndt))
    b2 = np.ascontiguousarray(band[256:256 + 2 * R].astype(ndt))

    halves = S // 2
    in_maps = []
    for c in range(N_CORES):
        b, half = divmod(c, 2)
        s0 = half * halves
        s1 = s0 + halves
        xc = np.zeros((XPAD, E), dtype=ndt)
        xc[R:R + ROWS] = x[b, s0:s1].astype(ndt)
        if s0 > 0:
            xc[0:R] = x[b, s0 - R:s0].astype(ndt)
        if s1 < S:
            xc[R + ROWS:R + ROWS + R] = x[b, s1:s1 + R].astype(ndt)
        in_maps.append({"x": _part_major(xc), "w1": w1s, "w2t": w2s,
                        "band01": b01, "band2": b2})
    return in_maps


def kernel(inputs: np.ndarray, input_weights: np.ndarray,
           output_weight: np.ndarray) -> np.ndarray:
    x = np.ascontiguousarray(np.asarray(inputs, dtype=np.float32))
    w1 = np.asarray(input_weights, dtype=np.float32)
    w2 = np.asarray(output_weight, dtype=np.float32)
    assert x.shape == (B, S, E) and w1.shape == (E, E) and w2.shape == (E, E)

    nc = _get_nc(DTYPE)
    in_maps = make_in_maps(x, w1, w2, DTYPE)
    res = run_bass_kernel_spmd(nc, in_maps, core_ids=list(range(N_CORES)))

    halves = S // 2
    out = np.empty((B, S, E), dtype=np.float32)
    for c in range(N_CORES):
        b, half = divmod(c, 2)
        s0 = half * halves
        out[b, s0:s0 + halves] = res.results[c]["out"]
    return out
